# revision 7
# baseline (speedup 1.0000x reference)
"""Causal single-head attention block on 8 TRN2 NeuronCores — fp8 version.

Problem: x[8, 2048, 1024] fp32; Wq/Wk/Wv [1024, 512]; bq/bk/bv [512].
  q = x@Wq + bq; k = x@Wk + bk; v = x@Wv + bv
  out = concat([x, softmax_causal(q k^T / sqrt(512)) @ v], axis=-1)

Sharding: data-parallel over batch — one batch element per core, no
collectives.

Per-core algorithm (S=2048, F=1024, D=512), all matmuls in fp8(e4m3)
with DoubleRow perf mode (256-row contraction pairs, ~1.8x fp32r rate):

  Phase A: stage all of x in SBUF (one big DMA per 512-row group), DMA
    it back out as the passthrough columns, convert to fp8, PE-transpose
    into xT8[f, s].  Projections: qT8/kT8[d, s] = W^T x^T with W-pair
    stationary; v8[s, d] = x W with xT8-pair stationary.  Biases folded
    in on the PSUM->SBUF fp8-quantizing copies (bv lands inside v8, so
    the attention output P@v8/rowsum == read + bv exactly).

  Phase C (per 512-wide q block, flash-style over k chunks):
    S^T strip [128k x 512q] = kT8-pair^T @ qT8-pair (2 DR matmuls);
    P^T = exp(S/sqrt(512))/64 via the activation bias (P fits fp8's
    range; the 1/64 cancels in the normalization), fp8 out;
    diagonal chunks masked with one affine_select;
    PV: psR[q-tile] += P-pair^T @ v8-pair (DR), rowsums via a 1-column
    matmul against ones reusing the same stationary P-pair;
    normalize by 1/rowsum on DVE, DMA per-block to the output.
"""

import numpy as np

import concourse.bass as bass
import concourse.bacc as bacc
import concourse.mybir as mybir
import concourse.tile as tile
from concourse.bass_utils import run_bass_kernel_spmd
from concourse.masks import make_identity

F32 = mybir.dt.float32
BF16 = mybir.dt.bfloat16
F8 = mybir.dt.float8e4
DR = mybir.MatmulPerfMode.DoubleRow

B, S, F, D = 8, 2048, 1024, 512
NSC = S // 128         # 16 s-chunks
NFC = F // 128         # 8 f-chunks (4 DR pairs)
NDC = D // 128         # 4 d-chunks (2 DR pairs)
NBLK = 4               # q blocks of 512
QB = 4                 # q-tiles per block
SCALE = 1.0 / np.sqrt(np.float32(D))
PBIAS = float(-np.log(64.0))   # P scaled by 1/64 to fit fp8e4 range


def build_program(reps=1):
    nc = bacc.Bacc("TRN2", target_bir_lowering=False, debug=False)

    x = nc.dram_tensor("x", [S, F], F32, kind="ExternalInput")
    Wq = nc.dram_tensor("Wq", [F, D], F32, kind="ExternalInput")
    bq = nc.dram_tensor("bq", [D], F32, kind="ExternalInput")
    Wk = nc.dram_tensor("Wk", [F, D], F32, kind="ExternalInput")
    bk = nc.dram_tensor("bk", [D], F32, kind="ExternalInput")
    Wv = nc.dram_tensor("Wv", [F, D], F32, kind="ExternalInput")
    bv = nc.dram_tensor("bv", [D], F32, kind="ExternalInput")
    out = nc.dram_tensor("out", [S, F + D], F32, kind="ExternalOutput")

    with tile.TileContext(nc) as tc:
        _emit(nc, tc, x, Wq, bq, Wk, bk, Wv, bv, out, reps=reps)
    nc.compile()
    return nc


def _emit(nc, tc, x, Wq, bq, Wk, bk, Wv, bv, out, reps=1):
    consts = tc.alloc_tile_pool(name="consts", bufs=1)
    persist = tc.alloc_tile_pool(name="persist", bufs=1)

    # ---- constants (input-independent, outside the rep loop) ----
    identb = consts.tile([128, 128], BF16, tag="identb", name="identb")
    make_identity(nc, identb[:, :])
    pbias = consts.tile([128, 1], F32, tag="pbias", name="pbias")
    nc.gpsimd.memset(pbias[:, :], PBIAS)

    # per-partition bias columns for q/k (bias varies along d = partitions)
    bq_c, bk_c = [], []
    for dc in range(NDC):
        for (src, lst, nm) in ((bq, bq_c, "bq"), (bk, bk_c, "bk")):
            t = consts.tile([128, 1], F32, tag=f"{nm}c{dc}", name=f"{nm}c{dc}")
            nc.gpsimd.dma_start(
                out=t[:, :],
                in_=src[dc * 128:(dc + 1) * 128].rearrange("(p o) -> p o", o=1))
            lst.append(t)
    # bv broadcast across partitions (varies along free axis)
    bv_bc = consts.tile([128, D], F32, tag="bv_bc", name="bv_bc")
    nc.gpsimd.dma_start(
        out=bv_bc[:, :],
        in_=bv.ap().unsqueeze(0).partition_broadcast(128).rearrange("p o f -> p (o f)"))

    # ---- persistent buffers (rewritten every rep) ----
    xs = persist.tile([128, NSC, F], F32, tag="xs", name="xs")          # 64KB/p
    xT8 = persist.tile([128, NFC, S], F8, tag="xT8", name="xT8")        # 16KB/p
    qT8 = persist.tile([128, NDC, S], F8, tag="qT8", name="qT8")        # 8KB/p
    kT8 = persist.tile([128, NDC, S], F8, tag="kT8", name="kT8")        # 8KB/p
    # v-cols 0:512 = x@Wv + bv; col 512 = 1.0 (rowsum column); rest unused pad
    v8 = persist.tile([128, NSC, 1024], F8, tag="v8", name="v8")        # 16KB/p
    w8 = {nm: persist.tile([128, NFC, D], F8, tag=f"w8{nm}", name=f"w8{nm}")
          for nm in ("q", "k", "v")}                                    # 12KB/p

    for _rep in range(reps):
        # =========== phase A: load, passthrough, transpose, project =========
        with tc.tile_pool(name="wstage", bufs=2) as wsp, \
             tc.tile_pool(name="x8p", bufs=2) as x8p, \
             tc.tile_pool(name="psx8", bufs=2, space="PSUM") as psx8p, \
             tc.tile_pool(name="psq", bufs=3, space="PSUM") as psqp:

            def load_w(Wsrc, nm):
                ws = wsp.tile([128, NFC, D], F32, tag="ws", name="ws")
                nc.sync.dma_start(
                    out=ws[:, :, :],
                    in_=Wsrc[:, :].rearrange("(c p) d -> p c d", p=128))
                nc.gpsimd.tensor_copy(out=w8[nm][:, :, :], in_=ws[:, :, :])

            def load_x_group(g):
                nc.sync.dma_start(
                    out=xs[:, 4 * g:4 * g + 4, :],
                    in_=x[g * 512:(g + 1) * 512, :].rearrange(
                        "(c p) f -> p c f", p=128))
                # passthrough half of the output, straight back out
                nc.scalar.dma_start(
                    out=out[g * 512:(g + 1) * 512, 0:F].rearrange(
                        "(c p) f -> p c f", p=128),
                    in_=xs[:, 4 * g:4 * g + 4, :])

            nc.gpsimd.memset(v8[:, :, 512:513], 1.0)

            # input DMA order on the sync ring: x g0 first (unblocks PE),
            # weights next (needed by first projections), rest of x after.
            load_x_group(0)
            load_w(Wq, "q")
            load_w(Wv, "v")
            load_x_group(1)
            load_w(Wk, "k")
            load_x_group(2)
            load_x_group(3)

            def transpose_chunk(sc):
                xbc = x8p.tile([128, F], BF16, tag="xbc", name="xbc")
                nc.gpsimd.tensor_copy(out=xbc[:, :], in_=xs[:, sc, :])
                pst = psx8p.tile([128, NFC, 128], BF16, tag="pst", name="pst")
                for j in range(NFC):
                    nc.tensor.transpose(
                        pst[:, j, :], xbc[:, j * 128:(j + 1) * 128], identb[:, :])
                if sc % 2 == 0:
                    nc.scalar.copy(
                        out=xT8[:, :, sc * 128:(sc + 1) * 128], in_=pst[:, :, :])
                else:
                    nc.vector.tensor_copy(
                        out=xT8[:, :, sc * 128:(sc + 1) * 128], in_=pst[:, :, :])

            def v_proj(sc):
                ps = psqp.tile([128, D], F32, tag="psq", name="psq")
                for fp in range(4):
                    nc.tensor.matmul(
                        ps[:, :],
                        lhsT=xT8[:, 2 * fp:2 * fp + 2, sc * 128:(sc + 1) * 128],
                        rhs=w8["v"][:, 2 * fp:2 * fp + 2, :],
                        start=(fp == 0), stop=(fp == 3), perf_mode=DR)
                nc.vector.tensor_tensor(
                    out=v8[:, sc, 0:512], in0=ps[:, :], in1=bv_bc[:, :],
                    op=mybir.AluOpType.add)

            def qk_strip(nm, dest, bcols, st):
                for dc in range(NDC):
                    ps = psqp.tile([128, 512], F32, tag="psq", name="psq")
                    for fp in range(4):
                        nc.tensor.matmul(
                            ps[:, :],
                            lhsT=w8[nm][:, 2 * fp:2 * fp + 2, dc * 128:(dc + 1) * 128],
                            rhs=xT8[:, 2 * fp:2 * fp + 2, st * 512:(st + 1) * 512],
                            start=(fp == 0), stop=(fp == 3), perf_mode=DR)
                    nc.vector.tensor_scalar_add(
                        out=dest[:, dc, st * 512:(st + 1) * 512],
                        in0=ps[:, :], scalar1=bcols[dc][:, :])

            # software pipeline: PE order = T(sc), v(sc-1), ... , strips(g-1)
            prev_sc = None
            pending_strip = None
            for g in range(4):
                for sc in range(4 * g, 4 * g + 4):
                    transpose_chunk(sc)
                    if prev_sc is not None:
                        v_proj(prev_sc)
                    prev_sc = sc
                if pending_strip is not None:
                    st = pending_strip
                    qk_strip("q", qT8, bq_c, st)
                    qk_strip("k", kT8, bk_c, st)
                pending_strip = g
            v_proj(prev_sc)
            qk_strip("q", qT8, bq_c, 3)
            qk_strip("k", kT8, bk_c, 3)

        # =========== phase C: causal attention, 512-wide q blocks ===========
        with tc.tile_pool(name="psS", bufs=2, space="PSUM") as psSp, \
             tc.tile_pool(name="psR", bufs=1, space="PSUM") as psRp, \
             tc.tile_pool(name="psL", bufs=1, space="PSUM") as psLp, \
             tc.tile_pool(name="P8p", bufs=3) as P8pool, \
             tc.tile_pool(name="ostage", bufs=2) as ostp, \
             tc.tile_pool(name="omisc", bufs=4) as omisc:

            for Bk in range(NBLK):
                npairs = 2 * Bk + 2
                psRa = [psRp.tile([128, 384], F32, tag=f"psRa{j}", name=f"psRa{j}")
                        for j in range(QB)]
                # [128, 2, 129] packs two q-tiles' (v-cols 384:512 + rowsum)
                # accumulators into one 2KB bank
                psRb = [psRp.tile([128, 2, 129], F32, tag=f"psRb{h}", name=f"psRb{h}")
                        for h in range(2)]

                def emit_pv(P8p, pair):
                    first = (pair == 0)
                    for j in range(QB):
                        # last pair whose chunks can touch q-tile j (chunks
                        # above the tile's diagonal are masked zeros — skip
                        # pairs that are entirely above it)
                        last_eff = min(npairs - 1, (4 * Bk + j) // 2)
                        if pair > last_eff:
                            continue
                        last = (pair == last_eff)
                        nc.tensor.matmul(
                            psRa[j][:, :],
                            lhsT=P8p[:, :, j * 128:(j + 1) * 128],
                            rhs=v8[:, 2 * pair:2 * pair + 2, 0:384],
                            start=first, stop=last, perf_mode=DR,
                            skip_group_check=True)
                        # v-cols 384:512 plus the ones column (-> rowsum at
                        # local col 128).  Two tiles share a psRb bank, so
                        # start=True only on the bank's first matmul: a start
                        # marks the ENTIRE 2KB bank pending-zero.
                        nc.tensor.matmul(
                            psRb[j // 2][:, j % 2, :],
                            lhsT=P8p[:, :, j * 128:(j + 1) * 128],
                            rhs=v8[:, 2 * pair:2 * pair + 2, 384:513],
                            start=(first and j % 2 == 0), stop=last,
                            perf_mode=DR, skip_group_check=True)

                prev_pair = None
                for pair in range(npairs):
                    P8p = P8pool.tile([128, 2, 512], F8, tag="P8", name="P8")
                    for half in range(2):
                        kc = 2 * pair + half
                        # cols < off are strictly above the causal diagonal:
                        # skip computing them and let the mask zero-fill
                        off = max(0, 128 * (kc - 4 * Bk))
                        psSt = psSp.tile([128, 512], F32, tag="psS", name="psS")
                        for dp in range(2):
                            nc.tensor.matmul(
                                psSt[:, off:512],
                                lhsT=kT8[:, 2 * dp:2 * dp + 2, kc * 128:(kc + 1) * 128],
                                rhs=qT8[:, 2 * dp:2 * dp + 2,
                                        Bk * 512 + off:(Bk + 1) * 512],
                                start=(dp == 0), stop=(dp == 1), perf_mode=DR)
                        nc.scalar.activation(
                            out=P8p[:, half, off:512], in_=psSt[:, off:512],
                            func=mybir.ActivationFunctionType.Exp,
                            scale=float(SCALE), bias=pbias[:, :])
                        if kc >= 4 * Bk:
                            # cols < off are entirely above the diagonal: zero
                            if off > 0:
                                nc.gpsimd.memset(P8p[:, half, 0:off], 0.0)
                            # the 128-wide boundary region: keep where the
                            # local column (q - k_chunk_base) >= partition (k)
                            nc.gpsimd.affine_select(
                                out=P8p[:, half, off:off + 128],
                                in_=P8p[:, half, off:off + 128],
                                compare_op=mybir.AluOpType.is_ge, fill=0.0,
                                base=0, channel_multiplier=-1,
                                pattern=[[1, 128]])
                    if prev_pair is not None:
                        emit_pv(*prev_pair)
                    prev_pair = (P8p, pair)
                emit_pv(*prev_pair)

                ot = ostp.tile([128, QB, D], F32, tag="ot", name="ot")
                for j in range(QB):
                    rl = omisc.tile([128, 1], F32, tag="rl", name="rl")
                    nc.vector.reciprocal(rl[:, :], psRb[j // 2][:, j % 2, 128:129])
                    nc.vector.tensor_scalar(
                        out=ot[:, j, 0:384], in0=psRa[j][:, :], scalar1=rl[:, :],
                        scalar2=None, op0=mybir.AluOpType.mult)
                    nc.vector.tensor_scalar(
                        out=ot[:, j, 384:512], in0=psRb[j // 2][:, j % 2, 0:128],
                        scalar1=rl[:, :],
                        scalar2=None, op0=mybir.AluOpType.mult)
                nc.scalar.dma_start(
                    out=out[Bk * 512:(Bk + 1) * 512, F:F + D].rearrange(
                        "(c p) d -> p c d", p=128),
                    in_=ot[:, :, :])

    for p in (persist, consts):
        p.release()


_NC_CACHE = None


def _get_program():
    global _NC_CACHE
    if _NC_CACHE is None:
        _NC_CACHE = build_program()
    return _NC_CACHE


def kernel(**inputs):
    nc = _get_program()
    arrs = {k: np.ascontiguousarray(np.asarray(v, dtype=np.float32))
            for k, v in inputs.items()}
    in_maps = []
    for b in range(B):
        m = {"x": arrs["x"][b]}
        for k in ("Wq", "bq", "Wk", "bk", "Wv", "bv"):
            m[k] = arrs[k]
        in_maps.append(m)
    res = run_bass_kernel_spmd(nc, in_maps, core_ids=list(range(B)))
    return np.stack([res.results[b]["out"] for b in range(B)], axis=0)


# revision 9
# speedup vs baseline: 1.1846x; 1.1846x over previous
"""Causal single-head attention block on 8 TRN2 NeuronCores — fp8 version.

Problem: x[8, 2048, 1024] fp32; Wq/Wk/Wv [1024, 512]; bq/bk/bv [512].
  q = x@Wq + bq; k = x@Wk + bk; v = x@Wv + bv
  out = concat([x, softmax_causal(q k^T / sqrt(512)) @ v], axis=-1)

Sharding: data-parallel over batch — one batch element per core, no
collectives.

Per-core algorithm (S=2048, F=1024, D=512), all matmuls in fp8(e4m3)
with DoubleRow perf mode (256-row contraction pairs, ~1.8x fp32r rate):

  Phase A: stage all of x in SBUF (one big DMA per 512-row group), DMA
    it back out as the passthrough columns, convert to fp8, PE-transpose
    into xT8[f, s].  Projections: qT8/kT8[d, s] = W^T x^T with W-pair
    stationary; v8[s, d] = x W with xT8-pair stationary.  Biases folded
    in on the PSUM->SBUF fp8-quantizing copies (bv lands inside v8, so
    the attention output P@v8/rowsum == read + bv exactly).

  Phase C (per 512-wide q block, flash-style over k chunks):
    S^T strip [128k x 512q] = kT8-pair^T @ qT8-pair (2 DR matmuls);
    P^T = exp(S/sqrt(512))/64 via the activation bias (P fits fp8's
    range; the 1/64 cancels in the normalization), fp8 out;
    diagonal chunks masked with one affine_select;
    PV: psR[q-tile] += P-pair^T @ v8-pair (DR), rowsums via a 1-column
    matmul against ones reusing the same stationary P-pair;
    normalize by 1/rowsum on DVE, DMA per-block to the output.
"""

import numpy as np

import concourse.bass as bass
import concourse.bacc as bacc
import concourse.mybir as mybir
import concourse.tile as tile
from concourse.bass_utils import run_bass_kernel_spmd
from concourse.masks import make_identity

F32 = mybir.dt.float32
BF16 = mybir.dt.bfloat16
F8 = mybir.dt.float8e4
DR = mybir.MatmulPerfMode.DoubleRow

B, S, F, D = 8, 2048, 1024, 512
NSC = S // 128         # 16 s-chunks
NFC = F // 128         # 8 f-chunks (4 DR pairs)
NDC = D // 128         # 4 d-chunks (2 DR pairs)
NBLK = 4               # q blocks of 512
QB = 4                 # q-tiles per block
SCALE = 1.0 / np.sqrt(np.float32(D))
PBIAS = float(-np.log(64.0))   # P scaled by 1/64 to fit fp8e4 range


def build_program(reps=1):
    nc = bacc.Bacc("TRN2", target_bir_lowering=False, debug=False)

    x = nc.dram_tensor("x", [S, F], F32, kind="ExternalInput")
    Wq = nc.dram_tensor("Wq", [F, D], F32, kind="ExternalInput")
    bq = nc.dram_tensor("bq", [D], F32, kind="ExternalInput")
    Wk = nc.dram_tensor("Wk", [F, D], F32, kind="ExternalInput")
    bk = nc.dram_tensor("bk", [D], F32, kind="ExternalInput")
    Wv = nc.dram_tensor("Wv", [F, D], F32, kind="ExternalInput")
    bv = nc.dram_tensor("bv", [D], F32, kind="ExternalInput")
    out = nc.dram_tensor("out", [S, F + D], F32, kind="ExternalOutput")

    with tile.TileContext(nc) as tc:
        _emit(nc, tc, x, Wq, bq, Wk, bk, Wv, bv, out, reps=reps)
    nc.compile()
    return nc


def _emit(nc, tc, x, Wq, bq, Wk, bk, Wv, bv, out, reps=1):
    consts = tc.alloc_tile_pool(name="consts", bufs=1)
    persist = tc.alloc_tile_pool(name="persist", bufs=1)

    # ---- constants (input-independent, outside the rep loop) ----
    identb = consts.tile([128, 128], BF16, tag="identb", name="identb")
    make_identity(nc, identb[:, :])
    pbias = consts.tile([128, 1], F32, tag="pbias", name="pbias")
    nc.gpsimd.memset(pbias[:, :], PBIAS)

    # per-partition bias columns for q/k (bias varies along d = partitions)
    bq_c, bk_c = [], []
    for dc in range(NDC):
        for (src, lst, nm) in ((bq, bq_c, "bq"), (bk, bk_c, "bk")):
            t = consts.tile([128, 1], F32, tag=f"{nm}c{dc}", name=f"{nm}c{dc}")
            nc.gpsimd.dma_start(
                out=t[:, :],
                in_=src[dc * 128:(dc + 1) * 128].rearrange("(p o) -> p o", o=1))
            lst.append(t)
    # bv broadcast across partitions (varies along free axis)
    bv_bc = consts.tile([128, D], F32, tag="bv_bc", name="bv_bc")
    nc.gpsimd.dma_start(
        out=bv_bc[:, :],
        in_=bv.ap().unsqueeze(0).partition_broadcast(128).rearrange("p o f -> p (o f)"))

    # ---- persistent buffers (rewritten every rep) ----
    xs = persist.tile([128, NSC, F], F32, tag="xs", name="xs")          # 64KB/p
    xT8 = persist.tile([128, NFC, S], F8, tag="xT8", name="xT8")        # 16KB/p
    qT8 = persist.tile([128, NDC, S], F8, tag="qT8", name="qT8")        # 8KB/p
    kT8 = persist.tile([128, NDC, S], F8, tag="kT8", name="kT8")        # 8KB/p
    # v-cols 0:512 = x@Wv + bv; col 512 = 1.0 (rowsum column); rest unused pad
    v8 = persist.tile([128, NSC, 1024], F8, tag="v8", name="v8")        # 16KB/p
    w8 = {nm: persist.tile([128, NFC, D], F8, tag=f"w8{nm}", name=f"w8{nm}")
          for nm in ("q", "k", "v")}                                    # 12KB/p

    for _rep in range(reps):
        # =========== phase A: load, passthrough, transpose, project =========
        with tc.tile_pool(name="wstage", bufs=2) as wsp, \
             tc.tile_pool(name="x8p", bufs=2) as x8p, \
             tc.tile_pool(name="psx8", bufs=2, space="PSUM") as psx8p, \
             tc.tile_pool(name="psq", bufs=3, space="PSUM") as psqp:

            def load_w(Wsrc, nm):
                ws = wsp.tile([128, NFC, D], F32, tag="ws", name="ws")
                nc.sync.dma_start(
                    out=ws[:, :, :],
                    in_=Wsrc[:, :].rearrange("(c p) d -> p c d", p=128))
                nc.gpsimd.tensor_copy(out=w8[nm][:, :, :], in_=ws[:, :, :])

            def load_x_group(g):
                nc.sync.dma_start(
                    out=xs[:, 4 * g:4 * g + 4, :],
                    in_=x[g * 512:(g + 1) * 512, :].rearrange(
                        "(c p) f -> p c f", p=128))
                # passthrough half of the output, straight back out
                nc.scalar.dma_start(
                    out=out[g * 512:(g + 1) * 512, 0:F].rearrange(
                        "(c p) f -> p c f", p=128),
                    in_=xs[:, 4 * g:4 * g + 4, :])

            nc.gpsimd.memset(v8[:, :, 512:513], 1.0)

            # input DMA order on the sync ring: x g0 first (unblocks PE),
            # weights next (needed by first projections), rest of x after.
            load_x_group(0)
            load_w(Wq, "q")
            load_w(Wv, "v")
            load_x_group(1)
            load_w(Wk, "k")
            load_x_group(2)
            load_x_group(3)

            def transpose_chunk(sc):
                xbc = x8p.tile([128, F], BF16, tag="xbc", name="xbc")
                nc.gpsimd.tensor_copy(out=xbc[:, :], in_=xs[:, sc, :])
                pst = psx8p.tile([128, NFC, 128], BF16, tag="pst", name="pst")
                for j in range(NFC):
                    nc.tensor.transpose(
                        pst[:, j, :], xbc[:, j * 128:(j + 1) * 128], identb[:, :])
                if sc % 2 == 0:
                    nc.scalar.copy(
                        out=xT8[:, :, sc * 128:(sc + 1) * 128], in_=pst[:, :, :])
                else:
                    nc.vector.tensor_copy(
                        out=xT8[:, :, sc * 128:(sc + 1) * 128], in_=pst[:, :, :])

            def v_proj(sc):
                ps = psqp.tile([128, D], F32, tag="psq", name="psq")
                for fp in range(4):
                    nc.tensor.matmul(
                        ps[:, :],
                        lhsT=xT8[:, 2 * fp:2 * fp + 2, sc * 128:(sc + 1) * 128],
                        rhs=w8["v"][:, 2 * fp:2 * fp + 2, :],
                        start=(fp == 0), stop=(fp == 3), perf_mode=DR)
                nc.vector.tensor_tensor(
                    out=v8[:, sc, 0:512], in0=ps[:, :], in1=bv_bc[:, :],
                    op=mybir.AluOpType.add)

            def qk_strip(nm, dest, bcols, st):
                for dc in range(NDC):
                    ps = psqp.tile([128, 512], F32, tag="psq", name="psq")
                    for fp in range(4):
                        nc.tensor.matmul(
                            ps[:, :],
                            lhsT=w8[nm][:, 2 * fp:2 * fp + 2, dc * 128:(dc + 1) * 128],
                            rhs=xT8[:, 2 * fp:2 * fp + 2, st * 512:(st + 1) * 512],
                            start=(fp == 0), stop=(fp == 3), perf_mode=DR)
                    nc.vector.tensor_scalar_add(
                        out=dest[:, dc, st * 512:(st + 1) * 512],
                        in0=ps[:, :], scalar1=bcols[dc][:, :])

            # software pipeline: PE order = T(sc), v(sc-1), ... , strips(g-1)
            prev_sc = None
            pending_strip = None
            for g in range(4):
                for sc in range(4 * g, 4 * g + 4):
                    transpose_chunk(sc)
                    if prev_sc is not None:
                        v_proj(prev_sc)
                    prev_sc = sc
                if pending_strip is not None:
                    st = pending_strip
                    qk_strip("q", qT8, bq_c, st)
                    qk_strip("k", kT8, bk_c, st)
                pending_strip = g
            v_proj(prev_sc)
            qk_strip("q", qT8, bq_c, 3)
            qk_strip("k", kT8, bk_c, 3)

        # =========== phase C: causal attention, 512-wide q blocks ===========
        with tc.tile_pool(name="psS", bufs=2, space="PSUM") as psSp, \
             tc.tile_pool(name="psR", bufs=1, space="PSUM") as psRp, \
             tc.tile_pool(name="psL", bufs=1, space="PSUM") as psLp, \
             tc.tile_pool(name="P8p", bufs=3) as P8pool, \
             tc.tile_pool(name="ostage", bufs=2) as ostp, \
             tc.tile_pool(name="omisc", bufs=4) as omisc:

            for Bk in range(NBLK):
                npairs = 2 * Bk + 2
                psRa = [psRp.tile([128, 384], F32, tag=f"psRa{j}", name=f"psRa{j}")
                        for j in range(QB)]
                # [128, 2, 129] packs two q-tiles' (v-cols 384:512 + rowsum)
                # accumulators into one 2KB bank
                psRb = [psRp.tile([128, 2, 129], F32, tag=f"psRb{h}", name=f"psRb{h}")
                        for h in range(2)]

                def emit_pv(P8p, pair):
                    first = (pair == 0)
                    for j in range(QB):
                        # last pair whose chunks can touch q-tile j (chunks
                        # above the tile's diagonal are masked zeros — skip
                        # pairs that are entirely above it)
                        last_eff = min(npairs - 1, (4 * Bk + j) // 2)
                        if pair > last_eff:
                            continue
                        last = (pair == last_eff)
                        nc.tensor.matmul(
                            psRa[j][:, :],
                            lhsT=P8p[:, :, j * 128:(j + 1) * 128],
                            rhs=v8[:, 2 * pair:2 * pair + 2, 0:384],
                            start=first, stop=last, perf_mode=DR,
                            skip_group_check=True)
                        # v-cols 384:512 plus the ones column (-> rowsum at
                        # local col 128).  Two tiles share a psRb bank, so
                        # start=True only on the bank's first matmul: a start
                        # marks the ENTIRE 2KB bank pending-zero.
                        nc.tensor.matmul(
                            psRb[j // 2][:, j % 2, :],
                            lhsT=P8p[:, :, j * 128:(j + 1) * 128],
                            rhs=v8[:, 2 * pair:2 * pair + 2, 384:513],
                            start=(first and j % 2 == 0), stop=last,
                            perf_mode=DR, skip_group_check=True)

                prev_pair = None
                for pair in range(npairs):
                    P8p = P8pool.tile([128, 2, 512], F8, tag="P8", name="P8")
                    for half in range(2):
                        kc = 2 * pair + half
                        # cols < off are strictly above the causal diagonal:
                        # skip computing them and let the mask zero-fill
                        off = max(0, 128 * (kc - 4 * Bk))
                        psSt = psSp.tile([128, 512], F32, tag="psS", name="psS")
                        for dp in range(2):
                            nc.tensor.matmul(
                                psSt[:, off:512],
                                lhsT=kT8[:, 2 * dp:2 * dp + 2, kc * 128:(kc + 1) * 128],
                                rhs=qT8[:, 2 * dp:2 * dp + 2,
                                        Bk * 512 + off:(Bk + 1) * 512],
                                start=(dp == 0), stop=(dp == 1), perf_mode=DR)
                        nc.scalar.activation(
                            out=P8p[:, half, off:512], in_=psSt[:, off:512],
                            func=mybir.ActivationFunctionType.Exp,
                            scale=float(SCALE), bias=pbias[:, :])
                        if kc >= 4 * Bk:
                            # cols < off are entirely above the diagonal: zero
                            if off > 0:
                                nc.gpsimd.memset(P8p[:, half, 0:off], 0.0)
                            # the 128-wide boundary region: keep where the
                            # local column (q - k_chunk_base) >= partition (k)
                            nc.gpsimd.affine_select(
                                out=P8p[:, half, off:off + 128],
                                in_=P8p[:, half, off:off + 128],
                                compare_op=mybir.AluOpType.is_ge, fill=0.0,
                                base=0, channel_multiplier=-1,
                                pattern=[[1, 128]])
                    if prev_pair is not None:
                        emit_pv(*prev_pair)
                    prev_pair = (P8p, pair)
                emit_pv(*prev_pair)

                ot = ostp.tile([128, QB, D], F32, tag="ot", name="ot")
                for j in range(QB):
                    rl = omisc.tile([128, 1], F32, tag="rl", name="rl")
                    nc.vector.reciprocal(rl[:, :], psRb[j // 2][:, j % 2, 128:129])
                    nc.vector.tensor_scalar(
                        out=ot[:, j, 0:384], in0=psRa[j][:, :], scalar1=rl[:, :],
                        scalar2=None, op0=mybir.AluOpType.mult)
                    nc.vector.tensor_scalar(
                        out=ot[:, j, 384:512], in0=psRb[j // 2][:, j % 2, 0:128],
                        scalar1=rl[:, :],
                        scalar2=None, op0=mybir.AluOpType.mult)
                nc.scalar.dma_start(
                    out=out[Bk * 512:(Bk + 1) * 512, F:F + D].rearrange(
                        "(c p) d -> p c d", p=128),
                    in_=ot[:, :, :])

    for p in (persist, consts):
        p.release()


_NC_CACHE = None


def _get_program():
    global _NC_CACHE
    if _NC_CACHE is None:
        _NC_CACHE = build_program()
    return _NC_CACHE


def kernel(**inputs):
    nc = _get_program()
    arrs = {k: np.ascontiguousarray(np.asarray(v, dtype=np.float32))
            for k, v in inputs.items()}
    in_maps = []
    for b in range(B):
        m = {"x": arrs["x"][b]}
        for k in ("Wq", "bq", "Wk", "bk", "Wv", "bv"):
            m[k] = arrs[k]
        in_maps.append(m)
    res = run_bass_kernel_spmd(nc, in_maps, core_ids=list(range(B)))
    return np.stack([res.results[b]["out"] for b in range(B)], axis=0)
